# revision 12
# baseline (speedup 1.0000x reference)
"""Trainium2 Bass kernel for nn_BaseModel_74302934220896 (TuckER + possibility-codebook).

Contract: kernel(**inputs) takes FULL unsharded inputs (as in reference.setup_inputs())
and returns the full output tuple (tucker_logits [B,N] f32, possibility_score [B,N] f32).

Sharding (8 cores):
  - B (2048) -> 8 x 256 for head/relation/hr/codebook paths
  - N (20000) -> 8 x 2500 (padded to 2560) for tail features and the [B,N] score matmuls
  - BN0 statistics via a tiny AllReduce of per-shard (sum, sumsq)
  - ONE AllGather carries the per-core [WmT(raw); interT] shards (bf16); BN1 statistics
    are computed locally from the gathered full-B WmT.
  - inter = einsum(hrm, tanh(codebook[rel])) runs on TensorE: one tiny matmul per b row
    against a host-replicated per-row codebook tile (stationary operand), writing one
    column of interT each.
  - Wm = einsum(ha, rs@core) uses a (d,c)-ordered core so the c-contraction is a single
    big DVE broadcast-multiply + reduce per d-chunk.
"""

import sys

sys.path.insert(0, "/opt/trn_rl_repo")

import numpy as np
import ml_dtypes

import concourse.bass as bass
import concourse.bacc as bacc
import concourse.mybir as mybir
import concourse.tile as tile
from concourse.bass_utils import run_bass_kernel_spmd
from concourse.masks import make_identity

F32 = mybir.dt.float32
BF16 = mybir.dt.bfloat16
I32 = mybir.dt.int32
AF = mybir.ActivationFunctionType
ALU = mybir.AluOpType
AX = mybir.AxisListType

B, N, E, C, R2 = 2048, 20000, 512, 128, 474
NCORES = 8
BSH = B // NCORES            # 256 b rows per core (sharded paths)
NSH = N // NCORES            # 2500 tail rows per core
NPAD = 2560                  # padded to 5 groups of 512
NG = NPAD // 512             # 5 n-groups
NB_FULL = B // 128           # 16 b-tiles over full B
TEMP = 0.5
NEG = -1.0e30
EPS = 1e-5

_PROG_CACHE = {}


def build_program():
    nc = bacc.Bacc("TRN2", target_bir_lowering=False, debug=False,
                   num_devices=NCORES)

    # ---------------- DRAM I/O ----------------
    dI = lambda name, shape, dt=F32: nc.dram_tensor(name, shape, dt, kind="ExternalInput")
    headT = dI("headT", [E, BSH], BF16)                # sharded head_vector^T
    relT = dI("relT", [E, BSH], BF16)                  # sharded relation_vector^T
    tailT = dI("tailT", [E, NPAD], BF16)               # sharded+padded tail_vector^T
    cbrow = dI("cbrow", [C, BSH * C], BF16)            # per-b codebook [c, (b,d)], tanh input
    core2 = dI("core2", [C, C * C], BF16)              # core [e, (d,c)]  (d outer, c inner)

    hsw1 = dI("hsw1", [E, E], BF16); hsb1 = dI("hsb1", [128, 4])
    hsw2 = dI("hsw2", [128, 4, 128], BF16)
    rsw1 = dI("rsw1", [E, E], BF16); rsb1 = dI("rsb1", [128, 4])
    rsw2 = dI("rsw2", [128, 4, 128], BF16); rsb2 = dI("rsb2", [128, 1])
    tsw1 = dI("tsw1", [E, E], BF16); tsb1 = dI("tsb1", [128, 4])
    tsw2 = dI("tsw2", [128, 4, 128], BF16); tsb2 = dI("tsb2", [128, 1])
    taw1 = dI("taw1", [E, E], BF16); tab1 = dI("tab1", [128, 4])
    taw2 = dI("taw2", [128, 4, 128], BF16); tab2 = dI("tab2", [128, 1])
    hrw1 = dI("hrw1", [2 * E, 2 * C], BF16); hrb1 = dI("hrb1", [128, 2])
    hrw2 = dI("hrw2", [128, 2, 256], BF16); hrb2 = dI("hrb2", [128, 2])
    hrw3 = dI("hrw3", [128, 2, 128], BF16); hrb3 = dI("hrb3", [128, 1])
    bn0g = dI("bn0g", [128, 1]); bn0b = dI("bn0b", [128, 1])
    bn1g = dI("bn1g", [128, 1]); bn1b = dI("bn1b", [128, 1])

    tucker = nc.dram_tensor("tucker", [B, NSH], BF16, kind="ExternalOutput")
    poss = nc.dram_tensor("poss", [B, NSH], BF16, kind="ExternalOutput")

    with tile.TileContext(nc) as tc:
        with (
            tc.tile_pool(name="const", bufs=1) as constp,
            tc.tile_pool(name="w1p", bufs=8) as w1p,
            tc.tile_pool(name="w2p", bufs=1) as w2p,
            tc.tile_pool(name="cbp", bufs=1) as cbp,
            tc.tile_pool(name="corep", bufs=1) as corep,
            tc.tile_pool(name="wdc", bufs=2) as wdcp,
            tc.tile_pool(name="xt", bufs=4) as xtp,
            tc.tile_pool(name="h1", bufs=6) as h1p,
            tc.tile_pool(name="pers", bufs=1) as pers,
            tc.tile_pool(name="small", bufs=2) as smallp,
            tc.tile_pool(name="stage", bufs=2) as stagep,
            tc.tile_pool(name="ost", bufs=2) as ostp,
            tc.tile_pool(name="pp", bufs=3, space="PSUM") as ppp,
            tc.tile_pool(name="pq", bufs=2, space="PSUM") as pqp,
            tc.tile_pool(name="dram", bufs=1, space="DRAM") as dramp,
        ):
            ident = constp.tile([128, 128], F32)
            make_identity(nc, ident[:])

            def load_const(dram_t, shape, tag, dt=F32):
                t = constp.tile(shape, dt, tag=tag)
                nc.sync.dma_start(out=t[:], in_=dram_t[:])
                return t

            hsb1_s = load_const(hsb1, [128, 4], "c01")
            rsb1_s = load_const(rsb1, [128, 4], "c02")
            tsb1_s = load_const(tsb1, [128, 4], "c03")
            tab1_s = load_const(tab1, [128, 4], "c04")
            hrb1_s = load_const(hrb1, [128, 2], "c05")
            hrb2_s = load_const(hrb2, [128, 2], "c06")
            hrb3_s = load_const(hrb3, [128, 1], "c07")
            rsb2_s = load_const(rsb2, [128, 1], "c08")
            tsb2_s = load_const(tsb2, [128, 1], "c09")
            tab2_s = load_const(tab2, [128, 1], "c10")
            bn0g_s = load_const(bn0g, [128, 1], "c11")
            bn0b_s = load_const(bn0b, [128, 1], "c12")
            bn1g_s = load_const(bn1g, [128, 1], "c13")
            bn1b_s = load_const(bn1b, [128, 1], "c14")

            # w2 weights pre-laid out on host as [128 part, k chunk, 128 c]
            def load_w2(w, tag):
                t = w2p.tile([128, 4, 128], BF16, tag=tag)
                nc.sync.dma_start(out=t[:], in_=w[:])
                return t

            hsw2_s = load_w2(hsw2, "w2a")
            rsw2_s = load_w2(rsw2, "w2b")
            tsw2_s = load_w2(tsw2, "w2c")
            taw2_s = load_w2(taw2, "w2d")
            hrw3_s = w2p.tile([128, 2, 128], BF16, tag="w2e")
            nc.sync.dma_start(out=hrw3_s[:], in_=hrw3[:])
            hrw2_s = w2p.tile([128, 2, 256], BF16, tag="w2f")
            nc.sync.dma_start(out=hrw2_s[:], in_=hrw2[:])

            # per-row codebook table [c, (b,d)] and core [e, (d,c)];
            # DMAs issued later (on the scalar queue) so the small critical-path
            # loads come first.
            cb_t = cbp.tile([128, BSH * C], BF16)
            core2_s = corep.tile([128, C * C], BF16)

            # persistent tiles
            hsT_sh = pers.tile([128, BSH], F32)       # hs^T shard (pre-BN)
            tsT_s = pers.tile([128, NPAD], BF16)      # ts^T (+bias)
            tamT_s = pers.tile([128, NPAD], BF16)     # tam^T
            WmT_sh = pers.tile([128, BSH], BF16)      # Wm^T raw shard
            intT_sh = pers.tile([128, BSH], BF16)     # inter^T shard
            WmT_all = pers.tile([128, B], BF16)       # gathered Wm^T raw -> BN1-applied
            intT_all = pers.tile([128, B], BF16)      # gathered inter^T
            hrmT_bf = pers.tile([128, BSH], BF16)     # hrm^T (masked), inter rhs

            def load_w1(w1_dram, nk):
                w1_t = []
                for k in range(nk):
                    wt = w1p.tile([128, w1_dram.shape[1]], BF16, tag="w1")
                    nc.sync.dma_start(out=wt[:], in_=w1_dram[k * 128:(k + 1) * 128, :])
                    w1_t.append(wt)
                return w1_t

            def load_xt(xT_dram, x_col0, nb, nk):
                xt_t = []
                for k in range(nk):
                    xt = xtp.tile([128, nb], BF16, tag="xt")
                    nc.sync.dma_start(
                        out=xt[:], in_=xT_dram[k * 128:(k + 1) * 128,
                                               x_col0:x_col0 + nb])
                    xt_t.append(xt)
                return xt_t

            def mlp2_T(w1_t, b1_tile, w2_tile, xt_t, nb, out_ap, b2_tile,
                       out_copy_dve=False):
                """out_ap [128, nb] (SBUF) = (relu(x@w1+b1)@w2 (+b2))^T for nb<=512 cols."""
                w1_nk = len(w1_t)
                nm = w1_t[0].shape[1] // 128
                h1_t = []
                for m in range(nm):
                    ps = pqp.tile([128, nb], F32, tag="pq")
                    for k in range(w1_nk):
                        nc.tensor.matmul(ps[:], w1_t[k][:, m * 128:(m + 1) * 128],
                                         xt_t[k][:], start=(k == 0),
                                         stop=(k == w1_nk - 1))
                    h1 = h1p.tile([128, nb], BF16, tag="h1")
                    nc.scalar.activation(h1[:], ps[:], AF.Relu,
                                         bias=b1_tile[:, m:m + 1])
                    h1_t.append(h1)
                ps2 = pqp.tile([128, nb], F32, tag="pq")
                for m in range(nm):
                    nc.tensor.matmul(ps2[:], w2_tile[:, m, :], h1_t[m][:],
                                     start=(m == 0), stop=(m == nm - 1))
                if b2_tile is None:
                    if out_copy_dve:
                        nc.vector.tensor_copy(out_ap, ps2[:])
                    else:
                        nc.scalar.activation(out_ap, ps2[:], AF.Copy)
                else:
                    nc.scalar.activation(out_ap, ps2[:], AF.Identity,
                                         bias=b2_tile[:, 0:1])
                return h1_t

            # ---------------- head MLP (shard) + BN0 partial sums ----------------
            hsw1_t = load_w1(hsw1, 4)
            xt_head = load_xt(headT, 0, BSH, 4)
            mlp2_T(hsw1_t, hsb1_s, hsw2_s, xt_head, BSH, hsT_sh[:], None,
                   out_copy_dve=True)

            stats = smallp.tile([128, 2], F32, tag="stats")
            sq = smallp.tile([128, BSH], F32, tag="sq")
            nc.vector.tensor_reduce(stats[:, 0:1], hsT_sh[:], axis=AX.X, op=ALU.add)
            nc.vector.tensor_tensor(out=sq[:], in0=hsT_sh[:], in1=hsT_sh[:],
                                    op=ALU.mult)
            nc.vector.tensor_reduce(stats[:, 1:2], sq[:], axis=AX.X, op=ALU.add)
            ar_in = dramp.tile([128, 2], F32)
            ar_out = dramp.tile([128, 2], F32, addr_space="Shared")
            nc.sync.dma_start(out=ar_in[:], in_=stats[:])
            nc.gpsimd.collective_compute(
                "AllReduce", ALU.add,
                replica_groups=[list(range(NCORES))],
                ins=[ar_in[:]], outs=[ar_out[:]])

            for q in range(4):
                nc.scalar.dma_start(
                    out=cb_t[:, q * 8192:(q + 1) * 8192],
                    in_=cbrow[:, q * 8192:(q + 1) * 8192])
            for hh in range(2):
                nc.scalar.dma_start(
                    out=core2_s[:, hh * 8192:(hh + 1) * 8192],
                    in_=core2[:, hh * 8192:(hh + 1) * 8192])

            # ---------------- rel MLP (shard) -> rsT ----------------
            rsw1_t = load_w1(rsw1, 4)
            rsT_bf = smallp.tile([128, BSH], BF16, tag="rsTbf")
            xt_rel = load_xt(relT, 0, BSH, 4)
            mlp2_T(rsw1_t, rsb1_s, rsw2_s, xt_rel, BSH, rsT_bf[:], rsb2_s)

            # ---------------- hr MLP (shard) -> hraT -> hrm -> hrmT ----------------
            hr_w1 = load_w1(hrw1, 8)
            hr_x = []
            for k in range(4):
                xt = xtp.tile([128, BSH], BF16, tag="xt")
                nc.sync.dma_start(out=xt[:], in_=headT[k * 128:(k + 1) * 128, :])
                hr_x.append(xt)
            for k in range(4):
                xt = xtp.tile([128, BSH], BF16, tag="xt")
                nc.sync.dma_start(out=xt[:], in_=relT[k * 128:(k + 1) * 128, :])
                hr_x.append(xt)
            hr_h1 = []
            for m in range(2):
                ps = pqp.tile([128, BSH], F32, tag="pq")
                for k in range(8):
                    nc.tensor.matmul(ps[:], hr_w1[k][:, m * 128:(m + 1) * 128],
                                     hr_x[k][:], start=(k == 0), stop=(k == 7))
                h1 = h1p.tile([128, BSH], BF16, tag="h1")
                nc.scalar.activation(h1[:], ps[:], AF.Relu, bias=hrb1_s[:, m:m + 1])
                hr_h1.append(h1)
            hr_h2 = []
            for m in range(2):
                ps = pqp.tile([128, BSH], F32, tag="pq")
                for k in range(2):
                    nc.tensor.matmul(ps[:], hrw2_s[:, k, m * 128:(m + 1) * 128],
                                     hr_h1[k][:], start=(k == 0), stop=(k == 1))
                h2 = h1p.tile([128, BSH], BF16, tag="h1")
                nc.scalar.activation(h2[:], ps[:], AF.Relu, bias=hrb2_s[:, m:m + 1])
                hr_h2.append(h2)
            hraT = smallp.tile([128, BSH], F32, tag="hraT")
            ps3 = pqp.tile([128, BSH], F32, tag="pq")
            for k in range(2):
                nc.tensor.matmul(ps3[:], hrw3_s[:, k, :], hr_h2[k][:],
                                 start=(k == 0), stop=(k == 1))
            nc.scalar.activation(hraT[:], ps3[:], AF.Identity, bias=hrb3_s[:, 0:1])

            # ---------- soft top-10 mask helper ([128,128] f32 tile) ----------
            def topk_mask_mul(x_ap, out_ap):
                """out = sigmoid((x - thr10)/TEMP) * x"""
                m8 = smallp.tile([128, 8], F32, tag="m8")
                zap = smallp.tile([128, 128], F32, tag="zap")
                nc.vector.max(out=m8[:], in_=x_ap)
                nc.vector.match_replace(out=zap[:], in_to_replace=m8[:],
                                        in_values=x_ap, imm_value=NEG)
                nc.vector.max(out=m8[:], in_=zap[:])
                thr = smallp.tile([128, 1], F32, tag="thr")
                nc.vector.tensor_scalar_mul(thr[:], m8[:, 1:2], -1.0 / TEMP)
                mask = smallp.tile([128, 128], F32, tag="mask")
                nc.scalar.activation(mask[:], x_ap, AF.Sigmoid,
                                     bias=thr[:, 0:1], scale=1.0 / TEMP)
                nc.vector.tensor_mul(out_ap, mask[:], x_ap)

            # hra -> hrm -> hrmT (bf16)
            hrm_ps = pqp.tile([128, 512], F32, tag="pq")
            hra_sb = smallp.tile([128, 256], F32, tag="hra")
            for t in range(2):
                nc.tensor.transpose(hrm_ps[:, t * 128:(t + 1) * 128],
                                    hraT[:, t * 128:(t + 1) * 128], ident[:])
            nc.vector.tensor_copy(hra_sb[:], hrm_ps[:, 0:256])
            hrm_sb = smallp.tile([128, 256], F32, tag="hrm")
            for t in range(2):
                topk_mask_mul(hra_sb[:, t * 128:(t + 1) * 128],
                              hrm_sb[:, t * 128:(t + 1) * 128])
            hrm_ps2 = pqp.tile([128, 512], F32, tag="pq")
            for t in range(2):
                nc.tensor.transpose(hrm_ps2[:, t * 128:(t + 1) * 128],
                                    hrm_sb[:, t * 128:(t + 1) * 128], ident[:])
            nc.vector.tensor_copy(hrmT_bf[:], hrm_ps2[:, 0:256])

            # ---------------- tail MLP + mask for one n-group ----------------
            tsw1_t = load_w1(tsw1, 4)
            taw1_t = load_w1(taw1, 4)

            def tail_group(g):
                xt_g = load_xt(tailT, g * 512, 512, 4)
                mlp2_T(tsw1_t, tsb1_s, tsw2_s, xt_g, 512,
                       tsT_s[:, g * 512:(g + 1) * 512], tsb2_s)
                taT_g = stagep.tile([128, 512], F32, tag="taT")
                mlp2_T(taw1_t, tab1_s, taw2_s, xt_g, 512, taT_g[:], tab2_s)
                # transpose all 4 tiles into one psum tile, single evac
                tps = pqp.tile([128, 512], F32, tag="pq")
                for j in range(4):
                    nc.tensor.transpose(tps[:, j * 128:(j + 1) * 128],
                                        taT_g[:, j * 128:(j + 1) * 128], ident[:])
                ta_nt = stagep.tile([128, 512], F32, tag="tant")
                nc.vector.tensor_copy(ta_nt[:], tps[:])
                tam_nt = stagep.tile([128, 512], F32, tag="tamnt")
                for j in range(4):
                    topk_mask_mul(ta_nt[:, j * 128:(j + 1) * 128],
                                  tam_nt[:, j * 128:(j + 1) * 128])
                tps2 = pqp.tile([128, 512], F32, tag="pq")
                for j in range(4):
                    nc.tensor.transpose(tps2[:, j * 128:(j + 1) * 128],
                                        tam_nt[:, j * 128:(j + 1) * 128], ident[:])
                nc.vector.tensor_copy(tamT_s[:, g * 512:(g + 1) * 512], tps2[:])

            tail_group(0)
            tail_group(1)

            # ---------------- W matmuls + Wm einsum (4 quarter passes) ----------------
            # BN0 scale/shift from AllReduced sums
            sums = smallp.tile([128, 2], F32, tag="sums")
            nc.sync.dma_start(out=sums[:], in_=ar_out[:])
            mean = smallp.tile([128, 1], F32, tag="mean")
            var = smallp.tile([128, 1], F32, tag="var")
            tmp1 = smallp.tile([128, 1], F32, tag="tmp1")
            bn0_scale = smallp.tile([128, 1], F32, tag="bn0s")
            bn0_shift = smallp.tile([128, 1], F32, tag="bn0h")
            nc.vector.tensor_scalar_mul(mean[:], sums[:, 0:1], 1.0 / B)
            nc.vector.tensor_scalar_mul(var[:], sums[:, 1:2], 1.0 / B)
            nc.vector.tensor_mul(tmp1[:], mean[:], mean[:])
            nc.vector.tensor_sub(var[:], var[:], tmp1[:])
            nc.vector.tensor_scalar_add(var[:], var[:], EPS)
            nc.scalar.activation(bn0_scale[:], var[:], AF.Sqrt)
            nc.vector.reciprocal(bn0_scale[:], bn0_scale[:])
            nc.vector.tensor_mul(bn0_scale[:], bn0_scale[:], bn0g_s[:, 0:1])
            nc.vector.tensor_mul(tmp1[:], mean[:], bn0_scale[:])
            nc.vector.tensor_sub(bn0_shift[:], bn0b_s[:, 0:1], tmp1[:])

            # ha (shard) in [b, c] bf16 tiles
            haT_aff = smallp.tile([128, BSH], F32, tag="haT")
            nc.vector.tensor_scalar(haT_aff[:], hsT_sh[:], bn0_scale[:, 0:1],
                                    bn0_shift[:, 0:1], op0=ALU.mult, op1=ALU.add)
            ha_ps = pqp.tile([128, 512], F32, tag="pq")
            for t in range(2):
                nc.tensor.transpose(ha_ps[:, t * 128:(t + 1) * 128],
                                    haT_aff[:, t * 128:(t + 1) * 128], ident[:])
            ha_bf = smallp.tile([128, 256], BF16, tag="habf")
            nc.vector.tensor_copy(ha_bf[:], ha_ps[:, 0:256])

            # W = rs @ core  in [b, (d,c)] order, quarter passes of 32 d each
            QD = 16                       # d rows per pass
            QW = QD * C                   # 2048 free elems per pass
            Wm_bd = smallp.tile([128, 2, C], F32, tag="wmbd")  # [b(t), d]
            for t in range(2):
                for q in range(8):
                    wdc = wdcp.tile([128, QW], BF16, tag="wdc")
                    for blk2 in range(QW // 1024):
                        ps = ppp.tile([128, 1024], F32, tag="pp")
                        for h in range(2):
                            col0 = q * QW + blk2 * 1024 + h * 512
                            nc.tensor.matmul(
                                ps[:, h * 512:(h + 1) * 512],
                                rsT_bf[:, t * 128:(t + 1) * 128],
                                core2_s[:, col0:col0 + 512],
                                start=True, stop=True)
                        nc.scalar.activation(
                            wdc[:, blk2 * 1024:(blk2 + 1) * 1024], ps[:], AF.Copy)
                    # multiply by ha (broadcast over d), reduce over c
                    wdc3 = wdc[:].rearrange("p (d c) -> p d c", c=C)
                    nc.vector.tensor_tensor(
                        out=wdc3, in0=wdc3,
                        in1=ha_bf[:, None, t * 128:(t + 1) * 128]
                            .to_broadcast([128, QD, C]),
                        op=ALU.mult)
                    nc.vector.tensor_reduce(
                        Wm_bd[:, t, q * QD:(q + 1) * QD], wdc3,
                        axis=AX.X, op=ALU.add)
            # transpose Wm [b,d] -> WmT [d,b] (bf16)
            wm_ps = pqp.tile([128, 512], F32, tag="pq")
            for t in range(2):
                nc.tensor.transpose(wm_ps[:, t * 128:(t + 1) * 128],
                                    Wm_bd[:, t, :], ident[:])
            nc.vector.tensor_copy(WmT_sh[:], wm_ps[:, 0:256])

            # ---------------- tanh codebook (in place, chunked) ----------------
            for q in range(8):
                nc.scalar.activation(cb_t[:, q * 4096:(q + 1) * 4096],
                                     cb_t[:, q * 4096:(q + 1) * 4096], AF.Tanh)

            # ---------------- inter: per-row matmuls on TensorE ----------------
            ips = pqp.tile([128, 512], F32, tag="pq")
            for b in range(BSH):
                nc.tensor.matmul(ips[:, b:b + 1], cb_t[:, b * C:(b + 1) * C],
                                 hrmT_bf[:, b:b + 1], start=True, stop=True)
            nc.vector.tensor_copy(intT_sh[:], ips[:, 0:BSH])

            # ---------------- AllGather of [WmT_sh ; intT_sh] (bf16) ----------------
            ag_in = dramp.tile([2, 128, BSH], BF16)
            ag_out = dramp.tile([NCORES, 2, 128, BSH], BF16, addr_space="Shared")
            nc.sync.dma_start(out=ag_in[0], in_=WmT_sh[:])
            nc.sync.dma_start(out=ag_in[1], in_=intT_sh[:])
            nc.gpsimd.collective_compute(
                "AllGather", ALU.bypass,
                replica_groups=[list(range(NCORES))],
                ins=[ag_in.opt()], outs=[ag_out.opt()])
            nc.sync.dma_start(
                out=WmT_all[:],
                in_=ag_out[:, 0].rearrange("r d b -> d r b"))
            nc.sync.dma_start(
                out=intT_all[:],
                in_=ag_out[:, 1].rearrange("r d b -> d r b"))

            # ---------------- remaining tail groups (hide the gather) ----------------
            for g in range(1, NG):
                tail_group(g)

            # ---------------- BN1 on gathered WmT (full B) ----------------
            st6 = smallp.tile([128, 4, 6], F32, tag="sm6")
            for i in range(4):
                nc.vector.bn_stats(st6[:, i, :], WmT_all[:, i * 512:(i + 1) * 512])
            mv = smallp.tile([128, 2], F32, tag="sm2")
            nc.vector.bn_aggr(mv[:], st6[:])
            bn1_scale = smallp.tile([128, 1], F32, tag="bn1s")
            bn1_shift = smallp.tile([128, 1], F32, tag="bn1h")
            nc.vector.tensor_scalar_add(tmp1[:], mv[:, 1:2], EPS)
            nc.scalar.activation(bn1_scale[:], tmp1[:], AF.Sqrt)
            nc.vector.reciprocal(bn1_scale[:], bn1_scale[:])
            nc.vector.tensor_mul(bn1_scale[:], bn1_scale[:], bn1g_s[:, 0:1])
            nc.vector.tensor_mul(tmp1[:], mv[:, 0:1], bn1_scale[:])
            nc.vector.tensor_sub(bn1_shift[:], bn1b_s[:, 0:1], tmp1[:])
            nc.vector.tensor_scalar(WmT_all[:], WmT_all[:], bn1_scale[:, 0:1],
                                    bn1_shift[:, 0:1], op0=ALU.mult, op1=ALU.add)

            # ---------------- scores: bt-major, batched output DMA ----------------
            evac_i = 0

            def evac(out_ap, ps_ap):
                nonlocal evac_i
                evac_i += 1
                if evac_i % 2 == 0:
                    nc.scalar.activation(out_ap, ps_ap, AF.Copy)
                else:
                    nc.vector.tensor_copy(out_ap, ps_ap)

            for bt in range(NB_FULL):
                st = ostp.tile([128, 2 * NPAD], BF16, tag="ost")
                st3 = st[:].rearrange("p (x n) -> p x n", x=2)
                for g in range(NG):
                    ps = ppp.tile([128, 1024], F32, tag="pp")
                    nc.tensor.matmul(ps[:, 0:512],
                                     WmT_all[:, bt * 128:(bt + 1) * 128],
                                     tsT_s[:, g * 512:(g + 1) * 512],
                                     start=True, stop=True)
                    nc.tensor.matmul(ps[:, 512:1024],
                                     intT_all[:, bt * 128:(bt + 1) * 128],
                                     tamT_s[:, g * 512:(g + 1) * 512],
                                     start=True, stop=True)
                    evac(st3[:, :, g * 512:(g + 1) * 512],
                         ps[:].rearrange("p (x n) -> p x n", x=2))
                nc.sync.dma_start(out=tucker[bt * 128:(bt + 1) * 128, :],
                                  in_=st[:, 0:NSH])
                nc.sync.dma_start(out=poss[bt * 128:(bt + 1) * 128, :],
                                  in_=st[:, NPAD:NPAD + NSH])
    nc.finalize()
    return nc


# ---------------------------------------------------------------------------
# host side
# ---------------------------------------------------------------------------

def _to_np(x, dt=np.float32):
    return np.ascontiguousarray(np.asarray(x), dtype=dt)


def prepare_in_maps(inputs):
    bf = np.dtype(ml_dtypes.bfloat16)
    head = _to_np(inputs["head_vector"])        # [B, E]
    rel = _to_np(inputs["relation_vector"])     # [B, E]
    ridx = np.asarray(inputs["relation_index"]).astype(np.int64)
    tailv = _to_np(inputs["tail_vector"])       # [N, E]
    codebook = _to_np(inputs["codebook"])       # [R2, C, C]
    core = _to_np(inputs["core"])               # [C, C, C]

    # core reshaped to [e, (d, c)]: d outer, c inner
    core2_host = np.ascontiguousarray(
        core.transpose(0, 2, 1).reshape(C, C * C)).astype(bf)

    headT_full = np.ascontiguousarray(head.T).astype(bf)   # [E, B]
    relT_full = np.ascontiguousarray(rel.T).astype(bf)     # [E, B]
    tailT_full = np.ascontiguousarray(tailv.T).astype(bf)  # [E, N]

    def chunked_bias(b, nk):
        return np.ascontiguousarray(_to_np(b).reshape(nk, 128).T)

    wcast = lambda k: _to_np(inputs[k]).astype(bf)

    def w2_layout(w, nk, nc_):
        # [nk*128, nc_] -> [128, nk, nc_] with partition = row within chunk
        return np.ascontiguousarray(
            _to_np(w).reshape(nk, 128, nc_).transpose(1, 0, 2)).astype(bf)
    weights_common = {
        "hsw1": wcast("hsw1"), "hsb1": chunked_bias(inputs["hsb1"], 4),
        "hsw2": w2_layout(inputs["hsw2"], 4, 128),
        "rsw1": wcast("rsw1"), "rsb1": chunked_bias(inputs["rsb1"], 4),
        "rsw2": w2_layout(inputs["rsw2"], 4, 128), "rsb2": _to_np(inputs["rsb2"]).reshape(128, 1),
        "tsw1": wcast("tsw1"), "tsb1": chunked_bias(inputs["tsb1"], 4),
        "tsw2": w2_layout(inputs["tsw2"], 4, 128), "tsb2": _to_np(inputs["tsb2"]).reshape(128, 1),
        "taw1": wcast("taw1"), "tab1": chunked_bias(inputs["tab1"], 4),
        "taw2": w2_layout(inputs["taw2"], 4, 128), "tab2": _to_np(inputs["tab2"]).reshape(128, 1),
        "hrw1": wcast("hrw1"), "hrb1": chunked_bias(inputs["hrb1"], 2),
        "hrw2": w2_layout(inputs["hrw2"], 2, 256), "hrb2": chunked_bias(inputs["hrb2"], 2),
        "hrw3": w2_layout(inputs["hrw3"], 2, 128), "hrb3": _to_np(inputs["hrb3"]).reshape(128, 1),
        "bn0g": _to_np(inputs["bn0_g"]).reshape(128, 1),
        "bn0b": _to_np(inputs["bn0_b"]).reshape(128, 1),
        "bn1g": _to_np(inputs["bn1_g"]).reshape(128, 1),
        "bn1b": _to_np(inputs["bn1_b"]).reshape(128, 1),
        "core2": core2_host,
    }

    cb_bf = codebook.astype(bf)                 # [R2, c, d]
    in_maps = []
    for k in range(NCORES):
        b0 = k * BSH
        n0 = k * NSH
        tailT_k = np.zeros((E, NPAD), bf)
        tailT_k[:, :NSH] = tailT_full[:, n0:n0 + NSH]
        # per-row codebook: [c, (b, d)] with row b's matrix at cols b*C..(b+1)*C
        cbr = cb_bf[ridx[b0:b0 + BSH]]          # [BSH, c, d]
        cbr = np.ascontiguousarray(
            cbr.transpose(1, 0, 2).reshape(C, BSH * C))
        m = dict(weights_common)
        m["headT"] = np.ascontiguousarray(headT_full[:, b0:b0 + BSH])
        m["relT"] = np.ascontiguousarray(relT_full[:, b0:b0 + BSH])
        m["tailT"] = tailT_k
        m["cbrow"] = cbr
        in_maps.append(m)
    return in_maps


def assemble_outputs(results):
    tuckers, posses = [], []
    for k in range(NCORES):
        r = results[k]
        tuckers.append(np.asarray(r["tucker"]).astype(np.float32))
        posses.append(np.asarray(r["poss"]).astype(np.float32))
    tucker_full = np.concatenate(tuckers, axis=1)
    poss_full = np.concatenate(posses, axis=1)
    return tucker_full, poss_full


def kernel(**inputs):
    if "prog" not in _PROG_CACHE:
        _PROG_CACHE["prog"] = build_program()
    nc = _PROG_CACHE["prog"]
    in_maps = prepare_in_maps(inputs)
    res = run_bass_kernel_spmd(nc, in_maps, list(range(NCORES)))
    return assemble_outputs(res.results)


# revision 17
# speedup vs baseline: 1.0295x; 1.0295x over previous
"""Trainium2 Bass kernel for nn_BaseModel_74302934220896 (TuckER + possibility-codebook).

Contract: kernel(**inputs) takes FULL unsharded inputs (as in reference.setup_inputs())
and returns the full output tuple (tucker_logits [B,N] f32, possibility_score [B,N] f32).

Sharding (8 cores):
  - B (2048) -> 8 x 256 for head/relation/hr/codebook paths
  - N (20000) -> 8 x 2500 (padded to 2560) for tail features and the [B,N] score matmuls
  - BN0 statistics via a tiny AllReduce of per-shard (sum, sumsq)
  - ONE AllGather carries the per-core [WmT(raw); interT] shards (bf16); BN1 statistics
    are computed locally from the gathered full-B WmT.
  - inter = einsum(hrm, tanh(codebook[rel])) runs on TensorE: one tiny matmul per b row
    against a host-replicated per-row codebook tile (stationary operand), writing one
    column of interT each.
  - Wm = einsum(ha, rs@core) uses a (d,c)-ordered core so the c-contraction is a single
    big DVE broadcast-multiply + reduce per d-chunk.
"""

import sys

sys.path.insert(0, "/opt/trn_rl_repo")

import numpy as np
import ml_dtypes

import concourse.bass as bass
import concourse.bacc as bacc
import concourse.mybir as mybir
import concourse.tile as tile
from concourse.bass_utils import run_bass_kernel_spmd
from concourse.masks import make_identity

F32 = mybir.dt.float32
BF16 = mybir.dt.bfloat16
I32 = mybir.dt.int32
AF = mybir.ActivationFunctionType
ALU = mybir.AluOpType
AX = mybir.AxisListType

B, N, E, C, R2 = 2048, 20000, 512, 128, 474
NCORES = 8
BSH = B // NCORES            # 256 b rows per core (sharded paths)
NSH = N // NCORES            # 2500 tail rows per core
NPAD = 2560                  # padded to 5 groups of 512
NG = NPAD // 512             # 5 n-groups
NB_FULL = B // 128           # 16 b-tiles over full B
TEMP = 0.5
NEG = -1.0e30
EPS = 1e-5

_PROG_CACHE = {}


def build_program():
    nc = bacc.Bacc("TRN2", target_bir_lowering=False, debug=False,
                   num_devices=NCORES)

    # ---------------- DRAM I/O ----------------
    dI = lambda name, shape, dt=F32: nc.dram_tensor(name, shape, dt, kind="ExternalInput")
    headT = dI("headT", [128, 4, BSH], BF16)           # sharded head_vector^T (k-chunked)
    relT = dI("relT", [128, 4, BSH], BF16)             # sharded relation_vector^T (k-chunked)
    tailT = dI("tailT", [128, 4, NPAD], BF16)          # sharded+padded tail_vector^T (k-chunked)
    cbrow = dI("cbrow", [C, BSH * C], BF16)            # per-b codebook [c, (b,d)], tanh input
    core2 = dI("core2", [C, C * C], BF16)              # core [e, (d,c)]  (d outer, c inner)

    # packed weight tensors (host pre-laid-out; single contiguous DMA each)
    consts = dI("consts", [128, 28])                   # all biases + bn params
    w2all = dI("w2all", [128, 2816], BF16)             # hsw2|rsw2|tsw2|taw2|hrw3|hrw2
    hsw1 = dI("hsw1", [128, 4, E], BF16)
    rsw1 = dI("rsw1", [128, 4, E], BF16)
    tsw1 = dI("tsw1", [128, 4, E], BF16)
    taw1 = dI("taw1", [128, 4, E], BF16)
    hrw1 = dI("hrw1", [128, 8, 2 * C], BF16)

    tucker = nc.dram_tensor("tucker", [B, NSH], BF16, kind="ExternalOutput")
    poss = nc.dram_tensor("poss", [B, NSH], BF16, kind="ExternalOutput")

    with tile.TileContext(nc) as tc:
        with (
            tc.tile_pool(name="const", bufs=1) as constp,
            tc.tile_pool(name="w1p", bufs=2) as w1p,
            tc.tile_pool(name="w2p", bufs=1) as w2p,
            tc.tile_pool(name="cbp", bufs=1) as cbp,
            tc.tile_pool(name="corep", bufs=1) as corep,
            tc.tile_pool(name="wdc", bufs=2) as wdcp,
            tc.tile_pool(name="xt", bufs=2) as xtp,
            tc.tile_pool(name="h1", bufs=6) as h1p,
            tc.tile_pool(name="pers", bufs=1) as pers,
            tc.tile_pool(name="small", bufs=1) as smallp,
            tc.tile_pool(name="stage", bufs=1) as stagep,
            tc.tile_pool(name="ost", bufs=2) as ostp,
            tc.tile_pool(name="pp", bufs=3, space="PSUM") as ppp,
            tc.tile_pool(name="pq", bufs=2, space="PSUM") as pqp,
            tc.tile_pool(name="dram", bufs=1, space="DRAM") as dramp,
        ):
            ident = constp.tile([128, 128], F32)
            make_identity(nc, ident[:])

            cst = constp.tile([128, 28], F32, tag="cst")
            nc.sync.dma_start(out=cst[:], in_=consts[:])
            hsb1_s = cst[:, 0:4]; rsb1_s = cst[:, 4:8]
            tsb1_s = cst[:, 8:12]; tab1_s = cst[:, 12:16]
            hrb1_s = cst[:, 16:18]; hrb2_s = cst[:, 18:20]
            hrb3_s = cst[:, 20:21]; rsb2_s = cst[:, 21:22]
            tsb2_s = cst[:, 22:23]; tab2_s = cst[:, 23:24]
            bn0g_s = cst[:, 24:25]; bn0b_s = cst[:, 25:26]
            bn1g_s = cst[:, 26:27]; bn1b_s = cst[:, 27:28]

            w2a = w2p.tile([128, 2816], BF16, tag="w2all")
            nc.sync.dma_start(out=w2a[:], in_=w2all[:])
            _w2v = lambda o, k, c: w2a[:, o:o + k * c].rearrange(
                "p (k c) -> p k c", c=c)
            hsw2_s = _w2v(0, 4, 128)
            rsw2_s = _w2v(512, 4, 128)
            tsw2_s = _w2v(1024, 4, 128)
            taw2_s = _w2v(1536, 4, 128)
            hrw3_s = _w2v(2048, 2, 128)
            hrw2_s = _w2v(2304, 2, 256)

            # per-row codebook table [c, (b,d)] and core [e, (d,c)];
            # DMAs issued later (on the scalar queue) so the small critical-path
            # loads come first.
            cb_t = cbp.tile([128, BSH * C], BF16)
            core2_s = corep.tile([128, C * C], BF16)

            # persistent tiles
            hsT_sh = pers.tile([128, BSH], F32)       # hs^T shard (pre-BN)
            tsT_s = pers.tile([128, NPAD], BF16)      # ts^T (+bias)
            tamT_s = pers.tile([128, NPAD], BF16)     # tam^T
            WmT_sh = pers.tile([128, BSH], BF16)      # Wm^T raw shard
            intT_sh = pers.tile([128, BSH], BF16)     # inter^T shard
            WmT_all = pers.tile([128, B], BF16)       # gathered Wm^T raw -> BN1-applied
            intT_all = pers.tile([128, B], BF16)      # gathered inter^T
            hrmT_bf = pers.tile([128, BSH], BF16)     # hrm^T (masked), inter rhs

            def load_w1(w1_dram, nk, tag="w1"):
                wid = w1_dram.shape[2]
                wt = w1p.tile([128, nk, wid], BF16, tag=tag)
                nc.sync.dma_start(out=wt[:], in_=w1_dram[:])
                return [wt[:, k, :] for k in range(nk)]

            def mlp2_T(w1_t, b1_tile, w2_tile, xt_t, nb, out_ap, b2_tile,
                       out_copy_dve=False):
                """out_ap [128, nb] (SBUF) = (relu(x@w1+b1)@w2 (+b2))^T for nb<=512 cols."""
                w1_nk = len(w1_t)
                nm = w1_t[0].shape[1] // 128
                h1_t = []
                for m in range(nm):
                    ps = pqp.tile([128, nb], F32, tag="pq")
                    for k in range(w1_nk):
                        nc.tensor.matmul(ps[:], w1_t[k][:, m * 128:(m + 1) * 128],
                                         xt_t[k][:], start=(k == 0),
                                         stop=(k == w1_nk - 1))
                    h1 = h1p.tile([128, nb], BF16, tag="h1")
                    nc.scalar.activation(h1[:], ps[:], AF.Relu,
                                         bias=b1_tile[:, m:m + 1])
                    h1_t.append(h1)
                ps2 = pqp.tile([128, nb], F32, tag="pq")
                for m in range(nm):
                    nc.tensor.matmul(ps2[:], w2_tile[:, m, :], h1_t[m][:],
                                     start=(m == 0), stop=(m == nm - 1))
                if b2_tile is None:
                    if out_copy_dve:
                        nc.vector.tensor_copy(out_ap, ps2[:])
                    else:
                        nc.scalar.activation(out_ap, ps2[:], AF.Copy)
                else:
                    nc.scalar.activation(out_ap, ps2[:], AF.Identity,
                                         bias=b2_tile[:, 0:1])
                return h1_t

            # ---------------- head MLP (shard) + BN0 partial sums ----------------
            hsw1_t = load_w1(hsw1, 4)
            headx = pers.tile([128, 4, BSH], BF16)
            nc.sync.dma_start(out=headx[:], in_=headT[:])
            xt_head = [headx[:, k, :] for k in range(4)]
            mlp2_T(hsw1_t, hsb1_s, hsw2_s, xt_head, BSH, hsT_sh[:], None,
                   out_copy_dve=True)

            stats = smallp.tile([128, 2], F32, tag="stats")
            sq = smallp.tile([128, BSH], F32, tag="sq")
            nc.vector.tensor_reduce(stats[:, 0:1], hsT_sh[:], axis=AX.X, op=ALU.add)
            nc.vector.tensor_tensor(out=sq[:], in0=hsT_sh[:], in1=hsT_sh[:],
                                    op=ALU.mult)
            nc.vector.tensor_reduce(stats[:, 1:2], sq[:], axis=AX.X, op=ALU.add)
            ar_in = dramp.tile([128, 2], F32)
            ar_out = dramp.tile([128, 2], F32, addr_space="Shared")
            nc.sync.dma_start(out=ar_in[:], in_=stats[:])
            nc.gpsimd.collective_compute(
                "AllReduce", ALU.add,
                replica_groups=[list(range(NCORES))],
                ins=[ar_in[:]], outs=[ar_out[:]])

            for q in range(4):
                nc.scalar.dma_start(
                    out=cb_t[:, q * 8192:(q + 1) * 8192],
                    in_=cbrow[:, q * 8192:(q + 1) * 8192])
            for hh in range(2):
                nc.scalar.dma_start(
                    out=core2_s[:, hh * 8192:(hh + 1) * 8192],
                    in_=core2[:, hh * 8192:(hh + 1) * 8192])

            # ---------------- rel MLP (shard) -> rsT ----------------
            rsw1_t = load_w1(rsw1, 4)
            relx = pers.tile([128, 4, BSH], BF16)
            nc.sync.dma_start(out=relx[:], in_=relT[:])
            rsT_bf = smallp.tile([128, BSH], BF16, tag="rsTbf")
            xt_rel = [relx[:, k, :] for k in range(4)]
            mlp2_T(rsw1_t, rsb1_s, rsw2_s, xt_rel, BSH, rsT_bf[:], rsb2_s)

            # ---------------- hr MLP (shard) -> hraT -> hrm -> hrmT ----------------
            hr_w1 = load_w1(hrw1, 8)
            hr_x = [headx[:, k, :] for k in range(4)] + \
                   [relx[:, k, :] for k in range(4)]
            hr_h1 = []
            for m in range(2):
                ps = pqp.tile([128, BSH], F32, tag="pq")
                for k in range(8):
                    nc.tensor.matmul(ps[:], hr_w1[k][:, m * 128:(m + 1) * 128],
                                     hr_x[k][:], start=(k == 0), stop=(k == 7))
                h1 = h1p.tile([128, BSH], BF16, tag="h1")
                nc.scalar.activation(h1[:], ps[:], AF.Relu, bias=hrb1_s[:, m:m + 1])
                hr_h1.append(h1)
            hr_h2 = []
            for m in range(2):
                ps = pqp.tile([128, BSH], F32, tag="pq")
                for k in range(2):
                    nc.tensor.matmul(ps[:], hrw2_s[:, k, m * 128:(m + 1) * 128],
                                     hr_h1[k][:], start=(k == 0), stop=(k == 1))
                h2 = h1p.tile([128, BSH], BF16, tag="h1")
                nc.scalar.activation(h2[:], ps[:], AF.Relu, bias=hrb2_s[:, m:m + 1])
                hr_h2.append(h2)
            hraT = smallp.tile([128, BSH], F32, tag="hraT")
            ps3 = pqp.tile([128, BSH], F32, tag="pq")
            for k in range(2):
                nc.tensor.matmul(ps3[:], hrw3_s[:, k, :], hr_h2[k][:],
                                 start=(k == 0), stop=(k == 1))
            nc.scalar.activation(hraT[:], ps3[:], AF.Identity, bias=hrb3_s[:, 0:1])

            # ---------- soft top-10 mask helper ([128,128] f32 tile) ----------
            def topk_mask_mul(x_ap, out_ap):
                """out = sigmoid((x - thr10)/TEMP) * x"""
                m8 = smallp.tile([128, 8], F32, tag="m8")
                zap = smallp.tile([128, 128], F32, tag="zap")
                nc.vector.max(out=m8[:], in_=x_ap)
                nc.vector.match_replace(out=zap[:], in_to_replace=m8[:],
                                        in_values=x_ap, imm_value=NEG)
                nc.vector.max(out=m8[:], in_=zap[:])
                thr = smallp.tile([128, 1], F32, tag="thr")
                nc.vector.tensor_scalar_mul(thr[:], m8[:, 1:2], -1.0 / TEMP)
                mask = smallp.tile([128, 128], F32, tag="mask")
                nc.scalar.activation(mask[:], x_ap, AF.Sigmoid,
                                     bias=thr[:, 0:1], scale=1.0 / TEMP)
                nc.vector.tensor_mul(out_ap, mask[:], x_ap)

            # hra -> hrm -> hrmT (bf16)
            hrm_ps = pqp.tile([128, 512], F32, tag="pq")
            hra_sb = smallp.tile([128, 256], F32, tag="hra")
            for t in range(2):
                nc.tensor.transpose(hrm_ps[:, t * 128:(t + 1) * 128],
                                    hraT[:, t * 128:(t + 1) * 128], ident[:])
            nc.vector.tensor_copy(hra_sb[:], hrm_ps[:, 0:256])
            hrm_sb = smallp.tile([128, 256], F32, tag="hrm")
            for t in range(2):
                topk_mask_mul(hra_sb[:, t * 128:(t + 1) * 128],
                              hrm_sb[:, t * 128:(t + 1) * 128])
            hrm_ps2 = pqp.tile([128, 512], F32, tag="pq")
            for t in range(2):
                nc.tensor.transpose(hrm_ps2[:, t * 128:(t + 1) * 128],
                                    hrm_sb[:, t * 128:(t + 1) * 128], ident[:])
            nc.vector.tensor_copy(hrmT_bf[:], hrm_ps2[:, 0:256])

            # ---------------- tail MLP + mask for one n-group ----------------
            tsw1_t = load_w1(tsw1, 4, tag="tsw1")
            taw1_t = load_w1(taw1, 4, tag="taw1")

            def tail_group(g):
                xt = xtp.tile([128, 4, 512], BF16, tag="xt")
                nc.sync.dma_start(out=xt[:], in_=tailT[:, :, g * 512:(g + 1) * 512])
                xt_g = [xt[:, k, :] for k in range(4)]
                mlp2_T(tsw1_t, tsb1_s, tsw2_s, xt_g, 512,
                       tsT_s[:, g * 512:(g + 1) * 512], tsb2_s)
                taT_g = stagep.tile([128, 512], F32, tag="taT")
                mlp2_T(taw1_t, tab1_s, taw2_s, xt_g, 512, taT_g[:], tab2_s)
                # transpose all 4 tiles into one psum tile, single evac
                tps = pqp.tile([128, 512], F32, tag="pq")
                for j in range(4):
                    nc.tensor.transpose(tps[:, j * 128:(j + 1) * 128],
                                        taT_g[:, j * 128:(j + 1) * 128], ident[:])
                ta_nt = stagep.tile([128, 512], F32, tag="tant")
                nc.vector.tensor_copy(ta_nt[:], tps[:])
                tam_nt = stagep.tile([128, 512], F32, tag="tamnt")
                for j in range(4):
                    topk_mask_mul(ta_nt[:, j * 128:(j + 1) * 128],
                                  tam_nt[:, j * 128:(j + 1) * 128])
                tps2 = pqp.tile([128, 512], F32, tag="pq")
                for j in range(4):
                    nc.tensor.transpose(tps2[:, j * 128:(j + 1) * 128],
                                        tam_nt[:, j * 128:(j + 1) * 128], ident[:])
                nc.vector.tensor_copy(tamT_s[:, g * 512:(g + 1) * 512], tps2[:])

            tail_group(0)
            tail_group(1)

            # ---------------- W matmuls + Wm einsum (4 quarter passes) ----------------
            # BN0 scale/shift from AllReduced sums
            sums = smallp.tile([128, 2], F32, tag="sums")
            nc.sync.dma_start(out=sums[:], in_=ar_out[:])
            mean = smallp.tile([128, 1], F32, tag="mean")
            var = smallp.tile([128, 1], F32, tag="var")
            tmp1 = smallp.tile([128, 1], F32, tag="tmp1")
            bn0_scale = smallp.tile([128, 1], F32, tag="bn0s")
            bn0_shift = smallp.tile([128, 1], F32, tag="bn0h")
            nc.vector.tensor_scalar_mul(mean[:], sums[:, 0:1], 1.0 / B)
            nc.vector.tensor_scalar_mul(var[:], sums[:, 1:2], 1.0 / B)
            nc.vector.tensor_mul(tmp1[:], mean[:], mean[:])
            nc.vector.tensor_sub(var[:], var[:], tmp1[:])
            nc.vector.tensor_scalar_add(var[:], var[:], EPS)
            nc.scalar.activation(bn0_scale[:], var[:], AF.Sqrt)
            nc.vector.reciprocal(bn0_scale[:], bn0_scale[:])
            nc.vector.tensor_mul(bn0_scale[:], bn0_scale[:], bn0g_s[:, 0:1])
            nc.vector.tensor_mul(tmp1[:], mean[:], bn0_scale[:])
            nc.vector.tensor_sub(bn0_shift[:], bn0b_s[:, 0:1], tmp1[:])

            # ha (shard) in [b, c] bf16 tiles
            haT_aff = smallp.tile([128, BSH], F32, tag="haT")
            nc.vector.tensor_scalar(haT_aff[:], hsT_sh[:], bn0_scale[:, 0:1],
                                    bn0_shift[:, 0:1], op0=ALU.mult, op1=ALU.add)
            ha_ps = pqp.tile([128, 512], F32, tag="pq")
            for t in range(2):
                nc.tensor.transpose(ha_ps[:, t * 128:(t + 1) * 128],
                                    haT_aff[:, t * 128:(t + 1) * 128], ident[:])
            ha_bf = smallp.tile([128, 256], BF16, tag="habf")
            nc.vector.tensor_copy(ha_bf[:], ha_ps[:, 0:256])

            # W = rs @ core  in [b, (d,c)] order, quarter passes of 32 d each
            QD = 16                       # d rows per pass
            QW = QD * C                   # 2048 free elems per pass
            Wm_bd = smallp.tile([128, 2, C], F32, tag="wmbd")  # [b(t), d]
            for t in range(2):
                for q in range(8):
                    wdc = wdcp.tile([128, QW], BF16, tag="wdc")
                    for blk2 in range(QW // 1024):
                        ps = ppp.tile([128, 1024], F32, tag="pp")
                        for h in range(2):
                            col0 = q * QW + blk2 * 1024 + h * 512
                            nc.tensor.matmul(
                                ps[:, h * 512:(h + 1) * 512],
                                rsT_bf[:, t * 128:(t + 1) * 128],
                                core2_s[:, col0:col0 + 512],
                                start=True, stop=True)
                        nc.scalar.activation(
                            wdc[:, blk2 * 1024:(blk2 + 1) * 1024], ps[:], AF.Copy)
                    # interleave one tanh chunk per pass so tanh finishes with W
                    tq = t * 8 + q
                    nc.scalar.activation(cb_t[:, tq * 2048:(tq + 1) * 2048],
                                         cb_t[:, tq * 2048:(tq + 1) * 2048],
                                         AF.Tanh)
                    # multiply by ha (broadcast over d), reduce over c
                    wdc3 = wdc[:].rearrange("p (d c) -> p d c", c=C)
                    nc.vector.tensor_tensor(
                        out=wdc3, in0=wdc3,
                        in1=ha_bf[:, None, t * 128:(t + 1) * 128]
                            .to_broadcast([128, QD, C]),
                        op=ALU.mult)
                    nc.vector.tensor_reduce(
                        Wm_bd[:, t, q * QD:(q + 1) * QD], wdc3,
                        axis=AX.X, op=ALU.add)
            # transpose Wm [b,d] -> WmT [d,b] (bf16)
            wm_ps = pqp.tile([128, 512], F32, tag="pq")
            for t in range(2):
                nc.tensor.transpose(wm_ps[:, t * 128:(t + 1) * 128],
                                    Wm_bd[:, t, :], ident[:])
            nc.vector.tensor_copy(WmT_sh[:], wm_ps[:, 0:256])

            # ---------------- inter: per-row matmuls on TensorE ----------------
            ips = pqp.tile([128, 512], F32, tag="pq")
            for b in range(BSH):
                nc.tensor.matmul(ips[:, b:b + 1], cb_t[:, b * C:(b + 1) * C],
                                 hrmT_bf[:, b:b + 1], start=True, stop=True)
            nc.vector.tensor_copy(intT_sh[:], ips[:, 0:BSH])

            # ---------------- AllGather of [WmT_sh ; intT_sh] (bf16) ----------------
            ag_in = dramp.tile([2, 128, BSH], BF16)
            ag_out = dramp.tile([NCORES, 2, 128, BSH], BF16, addr_space="Shared")
            nc.sync.dma_start(out=ag_in[0], in_=WmT_sh[:])
            nc.sync.dma_start(out=ag_in[1], in_=intT_sh[:])
            nc.gpsimd.collective_compute(
                "AllGather", ALU.bypass,
                replica_groups=[list(range(NCORES))],
                ins=[ag_in.opt()], outs=[ag_out.opt()])
            nc.sync.dma_start(
                out=WmT_all[:],
                in_=ag_out[:, 0].rearrange("r d b -> d r b"))
            nc.sync.dma_start(
                out=intT_all[:],
                in_=ag_out[:, 1].rearrange("r d b -> d r b"))

            # ---------------- remaining tail groups (hide the gather) ----------------
            for g in range(1, NG):
                tail_group(g)

            # ---------------- BN1 on gathered WmT (full B) ----------------
            st6 = smallp.tile([128, 4, 6], F32, tag="sm6")
            for i in range(4):
                nc.vector.bn_stats(st6[:, i, :], WmT_all[:, i * 512:(i + 1) * 512])
            mv = smallp.tile([128, 2], F32, tag="sm2")
            nc.vector.bn_aggr(mv[:], st6[:])
            bn1_scale = smallp.tile([128, 1], F32, tag="bn1s")
            bn1_shift = smallp.tile([128, 1], F32, tag="bn1h")
            nc.vector.tensor_scalar_add(tmp1[:], mv[:, 1:2], EPS)
            nc.scalar.activation(bn1_scale[:], tmp1[:], AF.Sqrt)
            nc.vector.reciprocal(bn1_scale[:], bn1_scale[:])
            nc.vector.tensor_mul(bn1_scale[:], bn1_scale[:], bn1g_s[:, 0:1])
            nc.vector.tensor_mul(tmp1[:], mv[:, 0:1], bn1_scale[:])
            nc.vector.tensor_sub(bn1_shift[:], bn1b_s[:, 0:1], tmp1[:])
            nc.vector.tensor_scalar(WmT_all[:], WmT_all[:], bn1_scale[:, 0:1],
                                    bn1_shift[:, 0:1], op0=ALU.mult, op1=ALU.add)

            # ---------------- scores: bt-major, batched output DMA ----------------
            evac_i = 0

            def evac(out_ap, ps_ap):
                nonlocal evac_i
                evac_i += 1
                if evac_i % 2 == 0:
                    nc.scalar.activation(out_ap, ps_ap, AF.Copy)
                else:
                    nc.vector.tensor_copy(out_ap, ps_ap)

            for bt in range(NB_FULL):
                st = ostp.tile([128, 2 * NPAD], BF16, tag="ost")
                st3 = st[:].rearrange("p (x n) -> p x n", x=2)
                for g in range(NG):
                    ps = ppp.tile([128, 1024], F32, tag="pp")
                    nc.tensor.matmul(ps[:, 0:512],
                                     WmT_all[:, bt * 128:(bt + 1) * 128],
                                     tsT_s[:, g * 512:(g + 1) * 512],
                                     start=True, stop=True)
                    nc.tensor.matmul(ps[:, 512:1024],
                                     intT_all[:, bt * 128:(bt + 1) * 128],
                                     tamT_s[:, g * 512:(g + 1) * 512],
                                     start=True, stop=True)
                    evac(st3[:, :, g * 512:(g + 1) * 512],
                         ps[:].rearrange("p (x n) -> p x n", x=2))
                    if g == 1:
                        nc.sync.dma_start(
                            out=tucker[bt * 128:(bt + 1) * 128, 0:1024],
                            in_=st[:, 0:1024])
                        nc.sync.dma_start(
                            out=poss[bt * 128:(bt + 1) * 128, 0:1024],
                            in_=st[:, NPAD:NPAD + 1024])
                nc.sync.dma_start(out=tucker[bt * 128:(bt + 1) * 128, 1024:NSH],
                                  in_=st[:, 1024:NSH])
                nc.sync.dma_start(out=poss[bt * 128:(bt + 1) * 128, 1024:NSH],
                                  in_=st[:, NPAD + 1024:NPAD + NSH])
    nc.finalize()
    return nc


# ---------------------------------------------------------------------------
# host side
# ---------------------------------------------------------------------------

def _to_np(x, dt=np.float32):
    return np.ascontiguousarray(np.asarray(x), dtype=dt)


def prepare_in_maps(inputs):
    bf = np.dtype(ml_dtypes.bfloat16)
    head = _to_np(inputs["head_vector"])        # [B, E]
    rel = _to_np(inputs["relation_vector"])     # [B, E]
    ridx = np.asarray(inputs["relation_index"]).astype(np.int64)
    tailv = _to_np(inputs["tail_vector"])       # [N, E]
    codebook = _to_np(inputs["codebook"])       # [R2, C, C]
    core = _to_np(inputs["core"])               # [C, C, C]

    # core reshaped to [e, (d, c)]: d outer, c inner
    core2_host = np.ascontiguousarray(
        core.transpose(0, 2, 1).reshape(C, C * C)).astype(bf)

    headT_full = np.ascontiguousarray(head.T).astype(bf)   # [E, B]
    relT_full = np.ascontiguousarray(rel.T).astype(bf)     # [E, B]
    tailT_full = tailv.T  # [E, N] float32

    def chunked_bias(b, nk):
        return np.ascontiguousarray(_to_np(b).reshape(nk, 128).T)

    def w1_layout(w, nk, wid):
        # [nk*128, wid] -> [128, nk, wid] with partition = row within chunk
        return np.ascontiguousarray(
            _to_np(w).reshape(nk, 128, wid).transpose(1, 0, 2)).astype(bf)

    def w2_layout(w, nk, nc_):
        return _to_np(w).reshape(nk, 128, nc_).transpose(1, 0, 2).reshape(
            128, nk * nc_)
    consts_host = np.concatenate([
        chunked_bias(inputs["hsb1"], 4), chunked_bias(inputs["rsb1"], 4),
        chunked_bias(inputs["tsb1"], 4), chunked_bias(inputs["tab1"], 4),
        chunked_bias(inputs["hrb1"], 2), chunked_bias(inputs["hrb2"], 2),
        _to_np(inputs["hrb3"]).reshape(128, 1),
        _to_np(inputs["rsb2"]).reshape(128, 1),
        _to_np(inputs["tsb2"]).reshape(128, 1),
        _to_np(inputs["tab2"]).reshape(128, 1),
        _to_np(inputs["bn0_g"]).reshape(128, 1),
        _to_np(inputs["bn0_b"]).reshape(128, 1),
        _to_np(inputs["bn1_g"]).reshape(128, 1),
        _to_np(inputs["bn1_b"]).reshape(128, 1),
    ], axis=1).astype(np.float32)
    w2all_host = np.ascontiguousarray(np.concatenate([
        w2_layout(inputs["hsw2"], 4, 128), w2_layout(inputs["rsw2"], 4, 128),
        w2_layout(inputs["tsw2"], 4, 128), w2_layout(inputs["taw2"], 4, 128),
        w2_layout(inputs["hrw3"], 2, 128), w2_layout(inputs["hrw2"], 2, 256),
    ], axis=1)).astype(bf)
    weights_common = {
        "consts": np.ascontiguousarray(consts_host),
        "w2all": w2all_host,
        "hsw1": w1_layout(inputs["hsw1"], 4, E),
        "rsw1": w1_layout(inputs["rsw1"], 4, E),
        "tsw1": w1_layout(inputs["tsw1"], 4, E),
        "taw1": w1_layout(inputs["taw1"], 4, E),
        "hrw1": w1_layout(inputs["hrw1"], 8, 2 * C),
        "core2": core2_host,
    }

    cb_bf = codebook.astype(bf)                 # [R2, c, d]
    in_maps = []
    for k in range(NCORES):
        b0 = k * BSH
        n0 = k * NSH
        tailT_k = np.zeros((E, NPAD), np.float32)
        tailT_k[:, :NSH] = tailT_full[:, n0:n0 + NSH]
        tailT_k = np.ascontiguousarray(
            tailT_k.reshape(4, 128, NPAD).transpose(1, 0, 2)).astype(bf)
        # per-row codebook: [c, (b, d)] with row b's matrix at cols b*C..(b+1)*C
        cbr = cb_bf[ridx[b0:b0 + BSH]]          # [BSH, c, d]
        cbr = np.ascontiguousarray(
            cbr.transpose(1, 0, 2).reshape(C, BSH * C))
        m = dict(weights_common)
        m["headT"] = np.ascontiguousarray(
            headT_full[:, b0:b0 + BSH].reshape(4, 128, BSH).transpose(1, 0, 2))
        m["relT"] = np.ascontiguousarray(
            relT_full[:, b0:b0 + BSH].reshape(4, 128, BSH).transpose(1, 0, 2))
        m["tailT"] = tailT_k
        m["cbrow"] = cbr
        in_maps.append(m)
    return in_maps


def assemble_outputs(results):
    tuckers, posses = [], []
    for k in range(NCORES):
        r = results[k]
        tuckers.append(np.asarray(r["tucker"]).astype(np.float32))
        posses.append(np.asarray(r["poss"]).astype(np.float32))
    tucker_full = np.concatenate(tuckers, axis=1)
    poss_full = np.concatenate(posses, axis=1)
    return tucker_full, poss_full


def kernel(**inputs):
    if "prog" not in _PROG_CACHE:
        _PROG_CACHE["prog"] = build_program()
    nc = _PROG_CACHE["prog"]
    in_maps = prepare_in_maps(inputs)
    res = run_bass_kernel_spmd(nc, in_maps, list(range(NCORES)))
    return assemble_outputs(res.results)


# revision 22
# speedup vs baseline: 1.1531x; 1.1201x over previous
"""Trainium2 Bass kernel for nn_BaseModel_74302934220896 (TuckER + possibility-codebook).

Contract: kernel(**inputs) takes FULL unsharded inputs (as in reference.setup_inputs())
and returns the full output tuple (tucker_logits [B,N] f32, possibility_score [B,N] f32).

Sharding (8 cores):
  - B (2048) -> 8 x 256 for head/relation/hr/codebook paths
  - N (20000) -> 8 x 2500 (padded to 2560) for tail features and the [B,N] score matmuls
  - BN0 statistics via a tiny AllReduce of per-shard (sum, sumsq); collectives and their
    staging DMAs ride the (otherwise idle) gpsimd software-DGE queue
  - ONE AllGather carries the per-core [WmT(raw); interT] shards (bf16); BN1 statistics
    are computed locally from the gathered full-B WmT
  - inter = einsum(hrm, tanh(codebook[rel])) runs on TensorE: one tiny matmul per b row
    against a host-replicated per-row codebook tile (stationary operand), writing one
    column of interT each; issued BEFORE the Wm chain so it never waits on the AllReduce
  - Wm = einsum(ha, rs@core) uses a (d,c)-ordered core so the c-contraction is a single
    big DVE broadcast-multiply + reduce per d-chunk
  - tail groups are split into MLP part (PE/ACT, issued early) and mask part (DVE,
    issued after the gather kick) so the gather is requested as early as possible
"""

import sys

sys.path.insert(0, "/opt/trn_rl_repo")

import numpy as np
import ml_dtypes

import concourse.bass as bass
import concourse.bacc as bacc
import concourse.mybir as mybir
import concourse.tile as tile
from concourse.bass_utils import run_bass_kernel_spmd
from concourse.masks import make_identity

F32 = mybir.dt.float32
BF16 = mybir.dt.bfloat16
AF = mybir.ActivationFunctionType
ALU = mybir.AluOpType
AX = mybir.AxisListType

B, N, E, C, R2 = 2048, 20000, 512, 128, 474
NCORES = 8
BSH = B // NCORES            # 256 b rows per core (sharded paths)
NSH = N // NCORES            # 2500 tail rows per core
NPAD = 2560                  # padded to 5 groups of 512
NG = NPAD // 512             # 5 n-groups
NB_FULL = B // 128           # 16 b-tiles over full B
TEMP = 0.5
NEG = -1.0e30
EPS = 1e-5

_PROG_CACHE = {}


def build_program():
    nc = bacc.Bacc("TRN2", target_bir_lowering=False, debug=False,
                   num_devices=NCORES)

    # ---------------- DRAM I/O ----------------
    dI = lambda name, shape, dt=F32: nc.dram_tensor(name, shape, dt, kind="ExternalInput")
    headT = dI("headT", [128, 4, BSH], BF16)           # sharded head_vector^T (k-chunked)
    relT = dI("relT", [128, 4, BSH], BF16)             # sharded relation_vector^T (k-chunked)
    tailT = dI("tailT", [128, 4, NPAD], BF16)          # sharded+padded tail_vector^T (k-chunked)
    cbrow = dI("cbrow", [C, BSH * C], BF16)            # per-b codebook [c, (b,d)], tanh input
    core2 = dI("core2", [C, C * C], BF16)              # core [e, (d,c)]  (d outer, c inner)

    consts = dI("consts", [128, 28])                   # all biases + bn params
    w2all = dI("w2all", [128, 2816], BF16)             # hsw2|rsw2|tsw2|taw2|hrw3|hrw2
    hsw1 = dI("hsw1", [128, 4, E], BF16)
    rsw1 = dI("rsw1", [128, 4, E], BF16)
    tsw1 = dI("tsw1", [128, 4, E], BF16)
    taw1 = dI("taw1", [128, 4, E], BF16)
    hrw1 = dI("hrw1", [128, 8, 2 * C], BF16)

    tucker = nc.dram_tensor("tucker", [B, NSH], BF16, kind="ExternalOutput")
    poss = nc.dram_tensor("poss", [B, NSH], BF16, kind="ExternalOutput")

    with tile.TileContext(nc) as tc:
        with (
            tc.tile_pool(name="const", bufs=1) as constp,
            tc.tile_pool(name="w1p", bufs=2) as w1p,
            tc.tile_pool(name="w2p", bufs=1) as w2p,
            tc.tile_pool(name="cbp", bufs=1) as cbp,
            tc.tile_pool(name="corep", bufs=1) as corep,
            tc.tile_pool(name="wdc", bufs=2) as wdcp,
            tc.tile_pool(name="xt", bufs=2) as xtp,
            tc.tile_pool(name="h1", bufs=6) as h1p,
            tc.tile_pool(name="pers", bufs=1) as pers,
            tc.tile_pool(name="small", bufs=1) as smallp,
            tc.tile_pool(name="taTp", bufs=3) as taTp,
            tc.tile_pool(name="maskp", bufs=1) as maskp,
            tc.tile_pool(name="ost", bufs=2) as ostp,
            tc.tile_pool(name="pp", bufs=3, space="PSUM") as ppp,
            tc.tile_pool(name="pq", bufs=2, space="PSUM") as pqp,
            tc.tile_pool(name="dram", bufs=1, space="DRAM") as dramp,
        ):
            ident = constp.tile([128, 128], F32)
            make_identity(nc, ident[:])

            # warm up the collective path with a dummy tiny AllReduce
            warm_in = dramp.tile([128, 2], F32)
            warm_out = dramp.tile([128, 2], F32, addr_space="Shared")
            nc.gpsimd.collective_compute(
                "AllReduce", ALU.add,
                replica_groups=[list(range(NCORES))],
                ins=[warm_in[:]], outs=[warm_out[:]])

            cst = constp.tile([128, 28], F32, tag="cst")
            nc.sync.dma_start(out=cst[:], in_=consts[:])
            hsb1_s = cst[:, 0:4]; rsb1_s = cst[:, 4:8]
            tsb1_s = cst[:, 8:12]; tab1_s = cst[:, 12:16]
            hrb1_s = cst[:, 16:18]; hrb2_s = cst[:, 18:20]
            hrb3_s = cst[:, 20:21]; rsb2_s = cst[:, 21:22]
            tsb2_s = cst[:, 22:23]; tab2_s = cst[:, 23:24]
            bn0g_s = cst[:, 24:25]; bn0b_s = cst[:, 25:26]
            bn1g_s = cst[:, 26:27]; bn1b_s = cst[:, 27:28]

            def load_w1(w1_dram, nk, tag="w1"):
                wid = w1_dram.shape[2]
                wt = w1p.tile([128, nk, wid], BF16, tag=tag)
                nc.sync.dma_start(out=wt[:], in_=w1_dram[:])
                return [wt[:, k, :] for k in range(nk)]

            # critical-path loads first
            hsw1_t = load_w1(hsw1, 4)
            headx = pers.tile([128, 4, BSH], BF16)
            nc.sync.dma_start(out=headx[:], in_=headT[:])

            w2a = w2p.tile([128, 2816], BF16, tag="w2all")
            nc.sync.dma_start(out=w2a[:], in_=w2all[:])
            _w2v = lambda o, k, c: w2a[:, o:o + k * c].rearrange(
                "p (k c) -> p k c", c=c)
            hsw2_s = _w2v(0, 4, 128)
            rsw2_s = _w2v(512, 4, 128)
            tsw2_s = _w2v(1024, 4, 128)
            taw2_s = _w2v(1536, 4, 128)
            hrw3_s = _w2v(2048, 2, 128)
            hrw2_s = _w2v(2304, 2, 256)

            # big off-critical loads on the scalar queue
            cb_t = cbp.tile([128, BSH * C], BF16)
            for q in range(4):
                nc.scalar.dma_start(
                    out=cb_t[:, q * 8192:(q + 1) * 8192],
                    in_=cbrow[:, q * 8192:(q + 1) * 8192])
            core2_s = corep.tile([128, C * C], BF16)
            for hh in range(2):
                nc.scalar.dma_start(
                    out=core2_s[:, hh * 8192:(hh + 1) * 8192],
                    in_=core2[:, hh * 8192:(hh + 1) * 8192])

            # persistent tiles
            hsT_sh = pers.tile([128, BSH], F32)       # hs^T shard (pre-BN)
            tsT_s = pers.tile([128, NPAD], BF16)      # ts^T (+bias)
            tamT_s = pers.tile([128, NPAD], BF16)     # tam^T
            WmT_sh = pers.tile([128, BSH], BF16)      # Wm^T raw shard
            intT_sh = pers.tile([128, BSH], BF16)     # inter^T shard
            WmT_all = pers.tile([128, B], BF16)       # gathered Wm^T raw -> BN1-applied
            intT_all = pers.tile([128, B], BF16)      # gathered inter^T
            hrmT_bf = pers.tile([128, BSH], BF16)     # hrm^T (masked), inter rhs

            def mlp2_T(w1_t, b1_ap, w2_tile, xt_t, nb, out_ap, b2_ap,
                       out_copy_dve=False):
                """out_ap [128, nb] (SBUF) = (relu(x@w1+b1)@w2 (+b2))^T for nb<=512."""
                w1_nk = len(w1_t)
                nm = w1_t[0].shape[1] // 128
                h1_t = []
                for m in range(nm):
                    ps = pqp.tile([128, nb], F32, tag="pq")
                    for k in range(w1_nk):
                        nc.tensor.matmul(ps[:], w1_t[k][:, m * 128:(m + 1) * 128],
                                         xt_t[k][:], start=(k == 0),
                                         stop=(k == w1_nk - 1))
                    h1 = h1p.tile([128, nb], BF16, tag="h1")
                    nc.scalar.activation(h1[:], ps[:], AF.Relu,
                                         bias=b1_ap[:, m:m + 1])
                    h1_t.append(h1)
                ps2 = pqp.tile([128, nb], F32, tag="pq")
                for m in range(nm):
                    nc.tensor.matmul(ps2[:], w2_tile[:, m, :], h1_t[m][:],
                                     start=(m == 0), stop=(m == nm - 1))
                if b2_ap is None:
                    if out_copy_dve:
                        nc.vector.tensor_copy(out_ap, ps2[:])
                    else:
                        nc.scalar.activation(out_ap, ps2[:], AF.Copy)
                else:
                    nc.scalar.activation(out_ap, ps2[:], AF.Identity,
                                         bias=b2_ap[:, 0:1])
                return h1_t

            # ---------------- head MLP (shard) + BN0 partial sums ----------------
            xt_head = [headx[:, k, :] for k in range(4)]
            mlp2_T(hsw1_t, hsb1_s, hsw2_s, xt_head, BSH, hsT_sh[:], None,
                   out_copy_dve=True)

            stats = smallp.tile([128, 2], F32, tag="stats")
            sq = smallp.tile([128, BSH], F32, tag="hraT")
            nc.vector.tensor_reduce(stats[:, 0:1], hsT_sh[:], axis=AX.X, op=ALU.add)
            nc.vector.tensor_tensor(out=sq[:], in0=hsT_sh[:], in1=hsT_sh[:],
                                    op=ALU.mult)
            nc.vector.tensor_reduce(stats[:, 1:2], sq[:], axis=AX.X, op=ALU.add)
            ar_in = dramp.tile([128, 2], F32)
            ar_out = dramp.tile([128, 2], F32, addr_space="Shared")
            nc.gpsimd.dma_start(out=ar_in[:], in_=stats[:])
            nc.gpsimd.collective_compute(
                "AllReduce", ALU.add,
                replica_groups=[list(range(NCORES))],
                ins=[ar_in[:]], outs=[ar_out[:]])
            sums = smallp.tile([128, 2], F32, tag="sums")
            nc.gpsimd.dma_start(out=sums[:], in_=ar_out[:])

            # ---------------- rel MLP (shard) -> rsT ----------------
            rsw1_t = load_w1(rsw1, 4)
            relx = pers.tile([128, 4, BSH], BF16)
            nc.sync.dma_start(out=relx[:], in_=relT[:])
            rsT_bf = smallp.tile([128, BSH], BF16, tag="rsTbf")
            xt_rel = [relx[:, k, :] for k in range(4)]
            mlp2_T(rsw1_t, rsb1_s, rsw2_s, xt_rel, BSH, rsT_bf[:], rsb2_s)

            # ---------------- hr MLP (shard) -> hraT -> hrm -> hrmT ----------------
            hr_w1 = load_w1(hrw1, 8)
            hr_x = [headx[:, k, :] for k in range(4)] + \
                   [relx[:, k, :] for k in range(4)]
            hr_h1 = []
            for m in range(2):
                ps = pqp.tile([128, BSH], F32, tag="pq")
                for k in range(8):
                    nc.tensor.matmul(ps[:], hr_w1[k][:, m * 128:(m + 1) * 128],
                                     hr_x[k][:], start=(k == 0), stop=(k == 7))
                h1 = h1p.tile([128, BSH], BF16, tag="h1")
                nc.scalar.activation(h1[:], ps[:], AF.Relu, bias=hrb1_s[:, m:m + 1])
                hr_h1.append(h1)
            hr_h2 = []
            for m in range(2):
                ps = pqp.tile([128, BSH], F32, tag="pq")
                for k in range(2):
                    nc.tensor.matmul(ps[:], hrw2_s[:, k, m * 128:(m + 1) * 128],
                                     hr_h1[k][:], start=(k == 0), stop=(k == 1))
                h2 = h1p.tile([128, BSH], BF16, tag="h1")
                nc.scalar.activation(h2[:], ps[:], AF.Relu, bias=hrb2_s[:, m:m + 1])
                hr_h2.append(h2)
            hraT = smallp.tile([128, BSH], F32, tag="hraT")
            ps3 = pqp.tile([128, BSH], F32, tag="pq")
            for k in range(2):
                nc.tensor.matmul(ps3[:], hrw3_s[:, k, :], hr_h2[k][:],
                                 start=(k == 0), stop=(k == 1))
            nc.scalar.activation(hraT[:], ps3[:], AF.Identity, bias=hrb3_s[:, 0:1])

            # ---------- soft top-10 mask helper ([128,128] f32 tile) ----------
            def topk_mask_mul(x_ap, out_ap):
                """out = sigmoid((x - thr10)/TEMP) * x"""
                m8 = smallp.tile([128, 8], F32, tag="m8")
                zap = smallp.tile([128, 128], F32, tag="zap")
                nc.vector.max(out=m8[:], in_=x_ap)
                nc.vector.match_replace(out=zap[:], in_to_replace=m8[:],
                                        in_values=x_ap, imm_value=NEG)
                nc.vector.max(out=m8[:], in_=zap[:])
                thr = smallp.tile([128, 1], F32, tag="thr")
                nc.vector.tensor_scalar_mul(thr[:], m8[:, 1:2], -1.0 / TEMP)
                nc.scalar.activation(zap[:], x_ap, AF.Sigmoid,
                                     bias=thr[:, 0:1], scale=1.0 / TEMP)
                nc.vector.tensor_mul(out_ap, zap[:], x_ap)

            # hra -> hrm -> hrmT (bf16)
            hrm_ps = pqp.tile([128, 512], F32, tag="pq")
            hra_sb = smallp.tile([128, 256], F32, tag="hra")
            for t in range(2):
                nc.tensor.transpose(hrm_ps[:, t * 128:(t + 1) * 128],
                                    hraT[:, t * 128:(t + 1) * 128], ident[:])
            nc.vector.tensor_copy(hra_sb[:], hrm_ps[:, 0:256])
            hrm_sb = hra_sb
            for t in range(2):
                topk_mask_mul(hra_sb[:, t * 128:(t + 1) * 128],
                              hrm_sb[:, t * 128:(t + 1) * 128])
            hrm_ps2 = pqp.tile([128, 512], F32, tag="pq")
            for t in range(2):
                nc.tensor.transpose(hrm_ps2[:, t * 128:(t + 1) * 128],
                                    hrm_sb[:, t * 128:(t + 1) * 128], ident[:])
            nc.vector.tensor_copy(hrmT_bf[:], hrm_ps2[:, 0:256])

            # ---------------- tail MLP / mask (split halves) ----------------
            tsw1_t = load_w1(tsw1, 4, tag="tsw1")
            taw1_t = load_w1(taw1, 4, tag="taw1")

            def tail_group_mlp(g):
                xt = xtp.tile([128, 4, 512], BF16, tag="xt")
                nc.sync.dma_start(out=xt[:], in_=tailT[:, :, g * 512:(g + 1) * 512])
                xt_g = [xt[:, k, :] for k in range(4)]
                mlp2_T(tsw1_t, tsb1_s, tsw2_s, xt_g, 512,
                       tsT_s[:, g * 512:(g + 1) * 512], tsb2_s)
                taT_g = taTp.tile([128, 512], F32, tag="taT")
                mlp2_T(taw1_t, tab1_s, taw2_s, xt_g, 512, taT_g[:], tab2_s)
                return taT_g

            def tail_group_mask(g, taT_g):
                tps = pqp.tile([128, 512], F32, tag="pq")
                for j in range(4):
                    nc.tensor.transpose(tps[:, j * 128:(j + 1) * 128],
                                        taT_g[:, j * 128:(j + 1) * 128], ident[:])
                ta_nt = maskp.tile([128, 512], F32, tag="tant")
                nc.vector.tensor_copy(ta_nt[:], tps[:])
                tam_nt = maskp.tile([128, 512], F32, tag="tamnt")
                for j in range(4):
                    topk_mask_mul(ta_nt[:, j * 128:(j + 1) * 128],
                                  tam_nt[:, j * 128:(j + 1) * 128])
                tps2 = pqp.tile([128, 512], F32, tag="pq")
                for j in range(4):
                    nc.tensor.transpose(tps2[:, j * 128:(j + 1) * 128],
                                        tam_nt[:, j * 128:(j + 1) * 128], ident[:])
                nc.vector.tensor_copy(tamT_s[:, g * 512:(g + 1) * 512], tps2[:])

            ta0 = tail_group_mlp(0)
            tail_group_mask(0, ta0)

            # ---------------- tanh codebook (in place, chunked) ----------------
            for q in range(8):
                nc.scalar.activation(cb_t[:, q * 4096:(q + 1) * 4096],
                                     cb_t[:, q * 4096:(q + 1) * 4096], AF.Tanh)

            # ---------------- inter: per-row matmuls on TensorE ----------------
            ips = pqp.tile([128, 512], F32, tag="pq")
            for b in range(BSH):
                nc.tensor.matmul(ips[:, b:b + 1], cb_t[:, b * C:(b + 1) * C],
                                 hrmT_bf[:, b:b + 1], start=True, stop=True)
            nc.vector.tensor_copy(intT_sh[:], ips[:, 0:BSH])

            ta1 = tail_group_mlp(1)
            tail_group_mask(1, ta1)

            # ---------------- BN0 scale/shift + ha ----------------
            mean = smallp.tile([128, 1], F32, tag="mean")
            var = smallp.tile([128, 1], F32, tag="var")
            tmp1 = smallp.tile([128, 1], F32, tag="tmp1")
            bn0_scale = smallp.tile([128, 1], F32, tag="bn0s")
            bn0_shift = smallp.tile([128, 1], F32, tag="bn0h")
            nc.vector.tensor_scalar_mul(mean[:], sums[:, 0:1], 1.0 / B)
            nc.vector.tensor_scalar_mul(var[:], sums[:, 1:2], 1.0 / B)
            nc.vector.tensor_mul(tmp1[:], mean[:], mean[:])
            nc.vector.tensor_sub(var[:], var[:], tmp1[:])
            nc.vector.tensor_scalar_add(var[:], var[:], EPS)
            nc.scalar.activation(bn0_scale[:], var[:], AF.Sqrt)
            nc.vector.reciprocal(bn0_scale[:], bn0_scale[:])
            nc.vector.tensor_mul(bn0_scale[:], bn0_scale[:], bn0g_s[:, 0:1])
            nc.vector.tensor_mul(tmp1[:], mean[:], bn0_scale[:])
            nc.vector.tensor_sub(bn0_shift[:], bn0b_s[:, 0:1], tmp1[:])

            haT_aff = hsT_sh
            nc.vector.tensor_scalar(haT_aff[:], hsT_sh[:], bn0_scale[:, 0:1],
                                    bn0_shift[:, 0:1], op0=ALU.mult, op1=ALU.add)
            ha_ps = pqp.tile([128, 512], F32, tag="pq")
            for t in range(2):
                nc.tensor.transpose(ha_ps[:, t * 128:(t + 1) * 128],
                                    haT_aff[:, t * 128:(t + 1) * 128], ident[:])
            ha_bf = smallp.tile([128, 256], BF16, tag="habf")
            nc.vector.tensor_copy(ha_bf[:], ha_ps[:, 0:256])

            # ---------------- W matmuls + Wm einsum ----------------
            QD = 16                       # d rows per pass
            QW = QD * C                   # 2048 free elems per pass
            Wm_bd = smallp.tile([128, 2, C], F32, tag="wmbd")  # [b(t), d]
            for t in range(2):
                for q in range(8):
                    wdc = wdcp.tile([128, QW], BF16, tag="wdc")
                    for blk2 in range(QW // 1024):
                        ps = ppp.tile([128, 1024], F32, tag="pp")
                        for h in range(2):
                            col0 = q * QW + blk2 * 1024 + h * 512
                            nc.tensor.matmul(
                                ps[:, h * 512:(h + 1) * 512],
                                rsT_bf[:, t * 128:(t + 1) * 128],
                                core2_s[:, col0:col0 + 512],
                                start=True, stop=True)
                        nc.scalar.activation(
                            wdc[:, blk2 * 1024:(blk2 + 1) * 1024], ps[:], AF.Copy)
                    wdc3 = wdc[:].rearrange("p (d c) -> p d c", c=C)
                    nc.vector.tensor_tensor(
                        out=wdc3, in0=wdc3,
                        in1=ha_bf[:, None, t * 128:(t + 1) * 128]
                            .to_broadcast([128, QD, C]),
                        op=ALU.mult)
                    nc.vector.tensor_reduce(
                        Wm_bd[:, t, q * QD:(q + 1) * QD], wdc3,
                        axis=AX.X, op=ALU.add)
            wm_ps = pqp.tile([128, 512], F32, tag="pq")
            for t in range(2):
                nc.tensor.transpose(wm_ps[:, t * 128:(t + 1) * 128],
                                    Wm_bd[:, t, :], ident[:])
            nc.vector.tensor_copy(WmT_sh[:], wm_ps[:, 0:256])

            # ---------------- AllGather of [WmT_sh ; intT_sh] (bf16) ----------------
            ag_in = dramp.tile([2, 128, BSH], BF16)
            ag_out = dramp.tile([NCORES, 2, 128, BSH], BF16, addr_space="Shared")
            nc.gpsimd.dma_start(out=ag_in[0], in_=WmT_sh[:])
            nc.gpsimd.dma_start(out=ag_in[1], in_=intT_sh[:])
            nc.gpsimd.collective_compute(
                "AllGather", ALU.bypass,
                replica_groups=[list(range(NCORES))],
                ins=[ag_in.opt()], outs=[ag_out.opt()])
            nc.gpsimd.dma_start(
                out=WmT_all[:],
                in_=ag_out[:, 0].rearrange("r d b -> d r b"))
            nc.gpsimd.dma_start(
                out=intT_all[:],
                in_=ag_out[:, 1].rearrange("r d b -> d r b"))

            # ---------------- remaining tail groups ----------------
            ta_tiles = []
            for g in range(2, NG):
                ta_tiles.append(tail_group_mlp(g))
            for g in range(2, NG):
                tail_group_mask(g, ta_tiles[g - 2])

            # ---------------- BN1 on gathered WmT (full B) ----------------
            st6 = smallp.tile([128, 4, 6], F32, tag="sm6")
            for i in range(4):
                nc.vector.bn_stats(st6[:, i, :], WmT_all[:, i * 512:(i + 1) * 512])
            mv = smallp.tile([128, 2], F32, tag="sm2")
            nc.vector.bn_aggr(mv[:], st6[:])
            bn1_scale = smallp.tile([128, 1], F32, tag="bn1s")
            bn1_shift = smallp.tile([128, 1], F32, tag="bn1h")
            nc.vector.tensor_scalar_add(tmp1[:], mv[:, 1:2], EPS)
            nc.scalar.activation(bn1_scale[:], tmp1[:], AF.Sqrt)
            nc.vector.reciprocal(bn1_scale[:], bn1_scale[:])
            nc.vector.tensor_mul(bn1_scale[:], bn1_scale[:], bn1g_s[:, 0:1])
            nc.vector.tensor_mul(tmp1[:], mv[:, 0:1], bn1_scale[:])
            nc.vector.tensor_sub(bn1_shift[:], bn1b_s[:, 0:1], tmp1[:])
            nc.vector.tensor_scalar(WmT_all[:], WmT_all[:], bn1_scale[:, 0:1],
                                    bn1_shift[:, 0:1], op0=ALU.mult, op1=ALU.add)

            # ---------------- scores: bt-major, batched + split output DMA ----------------
            evac_i = 0

            def evac(out_ap, ps_ap):
                nonlocal evac_i
                evac_i += 1
                if evac_i % 2 == 0:
                    nc.scalar.activation(out_ap, ps_ap, AF.Copy)
                else:
                    nc.vector.tensor_copy(out_ap, ps_ap)

            for bt in range(NB_FULL):
                st = ostp.tile([128, 2 * NPAD], BF16, tag="ost")
                st3 = st[:].rearrange("p (x n) -> p x n", x=2)
                for g in range(NG):
                    ps = ppp.tile([128, 1024], F32, tag="pp")
                    nc.tensor.matmul(ps[:, 0:512],
                                     WmT_all[:, bt * 128:(bt + 1) * 128],
                                     tsT_s[:, g * 512:(g + 1) * 512],
                                     start=True, stop=True)
                    nc.tensor.matmul(ps[:, 512:1024],
                                     intT_all[:, bt * 128:(bt + 1) * 128],
                                     tamT_s[:, g * 512:(g + 1) * 512],
                                     start=True, stop=True)
                    evac(st3[:, :, g * 512:(g + 1) * 512],
                         ps[:].rearrange("p (x n) -> p x n", x=2))
                    if g == 1:
                        nc.sync.dma_start(
                            out=tucker[bt * 128:(bt + 1) * 128, 0:1024],
                            in_=st[:, 0:1024])
                        nc.sync.dma_start(
                            out=poss[bt * 128:(bt + 1) * 128, 0:1024],
                            in_=st[:, NPAD:NPAD + 1024])
                nc.sync.dma_start(out=tucker[bt * 128:(bt + 1) * 128, 1024:NSH],
                                  in_=st[:, 1024:NSH])
                nc.sync.dma_start(out=poss[bt * 128:(bt + 1) * 128, 1024:NSH],
                                  in_=st[:, NPAD + 1024:NPAD + NSH])
    nc.finalize()
    return nc


# ---------------------------------------------------------------------------
# host side
# ---------------------------------------------------------------------------

def _to_np(x, dt=np.float32):
    return np.ascontiguousarray(np.asarray(x), dtype=dt)


def prepare_in_maps(inputs):
    bf = np.dtype(ml_dtypes.bfloat16)
    head = _to_np(inputs["head_vector"])        # [B, E]
    rel = _to_np(inputs["relation_vector"])     # [B, E]
    ridx = np.asarray(inputs["relation_index"]).astype(np.int64)
    tailv = _to_np(inputs["tail_vector"])       # [N, E]
    codebook = _to_np(inputs["codebook"])       # [R2, C, C]
    core = _to_np(inputs["core"])               # [C, C, C]

    core2_host = np.ascontiguousarray(
        core.transpose(0, 2, 1).reshape(C, C * C)).astype(bf)

    headT_full = np.ascontiguousarray(head.T).astype(bf)   # [E, B]
    relT_full = np.ascontiguousarray(rel.T).astype(bf)     # [E, B]
    tailT_full = tailv.T  # [E, N] float32

    def chunked_bias(b, nk):
        return np.ascontiguousarray(_to_np(b).reshape(nk, 128).T)

    def w1_layout(w, nk, wid):
        return np.ascontiguousarray(
            _to_np(w).reshape(nk, 128, wid).transpose(1, 0, 2)).astype(bf)

    def w2_layout(w, nk, nc_):
        return _to_np(w).reshape(nk, 128, nc_).transpose(1, 0, 2).reshape(
            128, nk * nc_)

    consts_host = np.concatenate([
        chunked_bias(inputs["hsb1"], 4), chunked_bias(inputs["rsb1"], 4),
        chunked_bias(inputs["tsb1"], 4), chunked_bias(inputs["tab1"], 4),
        chunked_bias(inputs["hrb1"], 2), chunked_bias(inputs["hrb2"], 2),
        _to_np(inputs["hrb3"]).reshape(128, 1),
        _to_np(inputs["rsb2"]).reshape(128, 1),
        _to_np(inputs["tsb2"]).reshape(128, 1),
        _to_np(inputs["tab2"]).reshape(128, 1),
        _to_np(inputs["bn0_g"]).reshape(128, 1),
        _to_np(inputs["bn0_b"]).reshape(128, 1),
        _to_np(inputs["bn1_g"]).reshape(128, 1),
        _to_np(inputs["bn1_b"]).reshape(128, 1),
    ], axis=1).astype(np.float32)
    w2all_host = np.ascontiguousarray(np.concatenate([
        w2_layout(inputs["hsw2"], 4, 128), w2_layout(inputs["rsw2"], 4, 128),
        w2_layout(inputs["tsw2"], 4, 128), w2_layout(inputs["taw2"], 4, 128),
        w2_layout(inputs["hrw3"], 2, 128), w2_layout(inputs["hrw2"], 2, 256),
    ], axis=1)).astype(bf)
    weights_common = {
        "consts": np.ascontiguousarray(consts_host),
        "w2all": w2all_host,
        "hsw1": w1_layout(inputs["hsw1"], 4, E),
        "rsw1": w1_layout(inputs["rsw1"], 4, E),
        "tsw1": w1_layout(inputs["tsw1"], 4, E),
        "taw1": w1_layout(inputs["taw1"], 4, E),
        "hrw1": w1_layout(inputs["hrw1"], 8, 2 * C),
        "core2": core2_host,
    }

    cb_bf = codebook.astype(bf)                 # [R2, c, d]
    in_maps = []
    for k in range(NCORES):
        b0 = k * BSH
        n0 = k * NSH
        tailT_k = np.zeros((E, NPAD), np.float32)
        tailT_k[:, :NSH] = tailT_full[:, n0:n0 + NSH]
        tailT_k = np.ascontiguousarray(
            tailT_k.reshape(4, 128, NPAD).transpose(1, 0, 2)).astype(bf)
        cbr = cb_bf[ridx[b0:b0 + BSH]]          # [BSH, c, d]
        cbr = np.ascontiguousarray(
            cbr.transpose(1, 0, 2).reshape(C, BSH * C))
        m = dict(weights_common)
        m["headT"] = np.ascontiguousarray(
            headT_full[:, b0:b0 + BSH].reshape(4, 128, BSH).transpose(1, 0, 2))
        m["relT"] = np.ascontiguousarray(
            relT_full[:, b0:b0 + BSH].reshape(4, 128, BSH).transpose(1, 0, 2))
        m["tailT"] = tailT_k
        m["cbrow"] = cbr
        in_maps.append(m)
    return in_maps


def assemble_outputs(results):
    tuckers, posses = [], []
    for k in range(NCORES):
        r = results[k]
        tuckers.append(np.asarray(r["tucker"]).astype(np.float32))
        posses.append(np.asarray(r["poss"]).astype(np.float32))
    tucker_full = np.concatenate(tuckers, axis=1)
    poss_full = np.concatenate(posses, axis=1)
    return tucker_full, poss_full


def kernel(**inputs):
    if "prog" not in _PROG_CACHE:
        _PROG_CACHE["prog"] = build_program()
    nc = _PROG_CACHE["prog"]
    in_maps = prepare_in_maps(inputs)
    res = run_bass_kernel_spmd(nc, in_maps, list(range(NCORES)))
    return assemble_outputs(res.results)
